# revision 3
# baseline (speedup 1.0000x reference)
"""Relative-position causal attention (B=4,H=16,S=1024,D=64) on 8 TRN2 NeuronCores.

Sharding: batch*heads (64) split 8 per core; pe tables replicated; no collectives.

Per (bh, q-block qt):
  scores[128, W] = (Q/8) @ (K + pe_k[0])^T in fp32 PSUM  (W=(qt+1)*128)
  qrel[128,33]   = (Q/8) @ (pe_k[r]-pe_k[0])^T
  band + causal mask injected via a DRAM stride trick: write qrel buckets
  1..32 as rows of stride 512 (bf16), read [128, 128/256] at row stride 511
  => per-row diagonal shift; the scratch's static padding supplies -1e9
  (upper triangle) and 0 (distant region, where pe_k[0] is already absorbed
  into K).
  exp on ACT (no max subtraction; fp32-safe) -> E bf16 + fp32 rowsum;
  p = E * (1/rowsum) in bf16; DMA the row to the packed lower-tri output.
  PV: for kt<=qt read P^T tiles via XBAR-transposed DMA of the packed output;
  matmul-accumulate P^T.T @ (V + pe_v[0]). Value band: skew-write the p row's
  diagonal vicinity to a stride-160 scratch, XBAR-transposed read, slice
  [0:32] -> one extra matmul against (pe_v[1..32]-pe_v[0]).

Host assembles full p_attn (upper tri = exact 0) and output; returns
(output, p_attn) like the reference.
"""

import numpy as np

B, H, S, D = 4, 16, 1024, 64
MAXP = 32
NCORES = 8
BH = B * H
BHPC = BH // NCORES
NQ = S // 128
W_OF = [(qt + 1) * 128 for qt in range(NQ)]
BASE_OF = [128 * 128 * (qt * (qt + 1) // 2) for qt in range(NQ)]
PPACK_PER_BH = 128 * 128 * (NQ * (NQ + 1) // 2)  # 589824

DPAD_ROWS, DPAD_W = 129, 512
DPAD_N = DPAD_ROWS * DPAD_W
VSCR_ST = 160
VSCR_ROWS = 130  # padded so [128,128] transposed reads stay in bounds
VSCR_N = VSCR_ROWS * VSCR_ST

_CACHE = {}


def _split_sync_waits(nc, max_waits=1):
    """This walrus build rejects >1 sync wait per instruction; move excess
    waits onto preceding NoOp carriers on the same engine."""
    from concourse import mybir

    for bb in nc.main_func.blocks:
        il = bb.instructions
        i = 0
        while i < len(il):
            ins = il[i]
            si = ins.sync_info
            if si is not None and si.on_wait is not None and len(si.on_wait) > max_waits:
                waits = list(si.on_wait)
                keep = waits[-max_waits:]
                excess = waits[:-max_waits]
                pos = i
                for j in range(0, len(excess), max_waits):
                    chunk = excess[j : j + max_waits]
                    nop = mybir.InstNoOp(name=f"{ins.name}_wsplit{j}", ins=[], outs=[])
                    nop.engine = ins.engine
                    nop.sync_info = mybir.SyncInfo(on_wait=chunk, on_update=[])
                    il.insert(pos, nop)
                    pos += 1
                    i += 1
                ins.sync_info = mybir.SyncInfo(
                    on_wait=keep, on_update=list(si.on_update or [])
                )
            i += 1


def _build_nc(debug=False):
    import contextlib

    import concourse.bass as bass
    import concourse.tile as tile
    from concourse import mybir

    dt = mybir.dt
    f32, bf16 = dt.float32, dt.bfloat16
    AP = bass.AP

    if debug:
        nc = bass.Bass(target_bir_lowering=False, debug=True)
    else:
        nc = bass.Bass()

    qt_in = nc.declare_dram_parameter("qt", [BHPC, D, S], f32, isOutput=False)
    kt_in = nc.declare_dram_parameter("kt", [BHPC, D, S], f32, isOutput=False)
    v_in = nc.declare_dram_parameter("v", [BHPC, S, D], f32, isOutput=False)
    pekd_in = nc.declare_dram_parameter("pekdT", [D, 33], f32, isOutput=False)
    pek0_in = nc.declare_dram_parameter("pek0", [D, 1], f32, isOutput=False)
    dv_in = nc.declare_dram_parameter("dv", [MAXP, D], f32, isOutput=False)
    pev0_in = nc.declare_dram_parameter("pev0", [128, D], f32, isOutput=False)
    dpad_in = nc.declare_dram_parameter("dpad_init", [DPAD_N], bf16, isOutput=False)
    zscr_in = nc.declare_dram_parameter("zscr", [VSCR_N], bf16, isOutput=False)

    ppack = nc.declare_dram_parameter(
        "p_pack", [BHPC * PPACK_PER_BH], bf16, isOutput=True
    )
    out_o = nc.declare_dram_parameter("out", [BHPC, S, D], f32, isOutput=True)

    dpad = nc.dram_tensor("dpad", [2 * DPAD_N], bf16)
    vscr0 = nc.dram_tensor("vscr0", [2 * VSCR_N], bf16)
    vscrp = nc.dram_tensor("vscrp", [2 * VSCR_N], bf16)

    with tile.TileContext(nc) as tc:
        ctx = contextlib.ExitStack()
        consts = ctx.enter_context(tc.tile_pool(name="consts", bufs=1))
        qk = ctx.enter_context(tc.tile_pool(name="qk", bufs=2))
        v0p = ctx.enter_context(tc.tile_pool(name="v0p", bufs=2))
        rows = ctx.enter_context(tc.tile_pool(name="rows", bufs=3))
        small = ctx.enter_context(tc.tile_pool(name="small", bufs=3))
        ptp = ctx.enter_context(tc.tile_pool(name="ptp", bufs=4))
        psum = ctx.enter_context(tc.tile_pool(name="psum", bufs=2, space="PSUM"))

        # one-time constants
        pekd32 = consts.tile([D, 33], f32)
        nc.sync.dma_start(out=pekd32[:], in_=pekd_in[:, :])
        pekd16 = consts.tile([D, 33], bf16)
        nc.vector.tensor_copy(out=pekd16[:], in_=pekd32[:])

        pek0 = consts.tile([D, 1], f32)
        nc.sync.dma_start(out=pek0[:], in_=pek0_in[:, :])

        dv32 = consts.tile([MAXP, D], f32)
        nc.sync.dma_start(out=dv32[:], in_=dv_in[:, :])
        dv16 = consts.tile([MAXP, D], bf16)
        nc.vector.tensor_copy(out=dv16[:], in_=dv32[:])

        pev0 = consts.tile([128, D], f32)
        nc.sync.dma_start(out=pev0[:], in_=pev0_in[:, :])

        for buf in range(2):
            nc.sync.dma_start(
                out=AP(tensor=dpad, offset=buf * DPAD_N, ap=[[DPAD_W, DPAD_ROWS], [1, DPAD_W]]),
                in_=AP(tensor=dpad_in, offset=0, ap=[[DPAD_W, DPAD_ROWS], [1, DPAD_W]]),
            )
            for scr in (vscr0, vscrp):
                nc.sync.dma_start(
                    out=AP(tensor=scr, offset=buf * VSCR_N, ap=[[VSCR_ST, VSCR_ROWS], [1, VSCR_ST]]),
                    in_=AP(tensor=zscr_in, offset=0, ap=[[VSCR_ST, VSCR_ROWS], [1, VSCR_ST]]),
                )

        step = 0
        for ibh in range(BHPC):
            q32 = qk.tile([D, S], f32, tag="q32")
            nc.sync.dma_start(out=q32[:], in_=qt_in[ibh, :, :])
            q16 = qk.tile([D, S], bf16, tag="q16")
            nc.vector.tensor_scalar_mul(q16[:], q32[:], 1.0 / np.sqrt(D))

            k32 = qk.tile([D, S], f32, tag="k32")
            nc.sync.dma_start(out=k32[:], in_=kt_in[ibh, :, :])
            k16 = qk.tile([D, S], bf16, tag="k16")
            nc.vector.tensor_scalar_add(k16[:], k32[:], pek0[:])

            v0 = []
            for kt in range(NQ):
                v32 = v0p.tile([128, D], f32, tag="v32")
                nc.sync.dma_start(out=v32[:], in_=v_in[ibh, kt * 128 : (kt + 1) * 128, :])
                v0t = v0p.tile([128, D], bf16, tag=f"v0_{kt}")
                nc.vector.tensor_add(v0t[:], v32[:], pev0[:])
                v0.append(v0t)

            for qt in range(NQ):
                W = W_OF[qt]
                dbuf = step % 2
                step += 1
                qsl = slice(qt * 128, (qt + 1) * 128)

                # scores
                sc = psum.tile([128, 1024], f32, tag="scores")
                for c0 in range(0, W, 512):
                    c1 = min(c0 + 512, W)
                    nc.tensor.matmul(
                        sc[:, c0:c1], q16[:, qsl], k16[:, c0:c1], start=True, stop=True
                    )
                qrel = psum.tile([128, 33], f32, tag="qrel")
                nc.tensor.matmul(qrel[:], q16[:, qsl], pekd16[:], start=True, stop=True)

                # band skew through DRAM
                qd = small.tile([128, MAXP], bf16, tag="qd")
                nc.vector.tensor_copy(out=qd[:], in_=qrel[:, 1:33])
                nc.sync.dma_start(
                    out=AP(tensor=dpad, offset=dbuf * DPAD_N + DPAD_W, ap=[[DPAD_W, 128], [1, MAXP]]),
                    in_=qd[:],
                )
                if qt == 0:
                    bb = small.tile([128, 128], bf16, tag="bb0")
                    nc.sync.dma_start(
                        out=bb[:],
                        in_=AP(tensor=dpad, offset=dbuf * DPAD_N + DPAD_W + 31, ap=[[DPAD_W - 1, 128], [1, 128]]),
                    )
                    nc.vector.tensor_add(sc[:, 0:128], sc[:, 0:128], bb[:])
                else:
                    bb = small.tile([128, 256], bf16, tag="bb")
                    nc.sync.dma_start(
                        out=bb[:],
                        in_=AP(tensor=dpad, offset=dbuf * DPAD_N + DPAD_W - 97, ap=[[DPAD_W - 1, 128], [1, 256]]),
                    )
                    pr = slice((qt - 1) * 128, (qt + 1) * 128)
                    nc.vector.tensor_add(sc[:, pr], sc[:, pr], bb[:])

                # softmax
                e16 = rows.tile([128, 1024], bf16, tag="e16")
                rsum = small.tile([128, 1], f32, tag="rsum")
                nc.scalar.activation(
                    out=e16[:, :W],
                    in_=sc[:, :W],
                    func=mybir.ActivationFunctionType.Exp,
                    accum_out=rsum[:],
                )
                rinv = small.tile([128, 1], f32, tag="rinv")
                nc.vector.reciprocal(rinv[:], rsum[:])
                p16 = rows.tile([128, 1024], bf16, tag="p16")
                nc.vector.tensor_scalar_mul(p16[:, :W], e16[:, :W], rinv[:])

                pbase = ibh * PPACK_PER_BH + BASE_OF[qt]
                nc.sync.dma_start(
                    out=AP(tensor=ppack, offset=pbase, ap=[[W, 128], [1, W]]),
                    in_=p16[:, :W],
                )

                # PV via transposed readback of packed p tiles
                opsum = psum.tile([128, D], f32, tag="outp")
                for kt in range(qt + 1):
                    pt = ptp.tile([128, 128], bf16, tag="pt")
                    nc.sync.dma_start(
                        out=pt[:],
                        in_=AP(tensor=ppack, offset=pbase + kt * 128, ap=[[W, 128], [1, 128]]),
                        transpose=True,
                    )
                    nc.tensor.matmul(
                        opsum[:], pt[:], v0[kt][:], start=(kt == 0), stop=False
                    )

                # value band: skew-write p row vicinity, transposed read
                bwT = small.tile([128, 128], bf16, tag="bwT")
                if qt == 0:
                    nc.sync.dma_start(
                        out=AP(tensor=vscr0, offset=dbuf * VSCR_N + VSCR_ST, ap=[[VSCR_ST, 128], [1, 128]]),
                        in_=p16[:, 0:128],
                    )
                    nc.sync.dma_start(
                        out=bwT[:],
                        in_=AP(tensor=vscr0, offset=dbuf * VSCR_N + VSCR_ST - 31, ap=[[VSCR_ST + 1, 128], [1, 128]]),
                        transpose=True,
                    )
                else:
                    nc.sync.dma_start(
                        out=AP(tensor=vscrp, offset=dbuf * VSCR_N, ap=[[VSCR_ST, 128], [1, VSCR_ST]]),
                        in_=p16[:, (qt - 1) * 128 + 96 : (qt + 1) * 128],
                    )
                    nc.sync.dma_start(
                        out=bwT[:],
                        in_=AP(tensor=vscrp, offset=dbuf * VSCR_N + 1, ap=[[VSCR_ST + 1, 128], [1, 128]]),
                        transpose=True,
                    )
                nc.tensor.matmul(opsum[:], bwT[0:MAXP, :], dv16[:], start=False, stop=True)

                osb = small.tile([128, D], f32, tag="osb")
                nc.any.tensor_copy(out=osb[:], in_=opsum[:])
                nc.sync.dma_start(out=out_o[ibh, qsl, :], in_=osb[:])

        ctx.close()

    if not debug:
        # only needed for the walrus compile path; CoreSim chokes on
        # instructions inserted outside the build hooks
        _split_sync_waits(nc)
    return nc


def host_prep(query, key, value, pe_k, pe_v):
    import ml_dtypes

    q = query.reshape(BH, S, D)
    k = key.reshape(BH, S, D)
    v = np.ascontiguousarray(value.reshape(BH, S, D)).astype(np.float32)
    qT = np.ascontiguousarray(q.transpose(0, 2, 1)).astype(np.float32)
    kT = np.ascontiguousarray(k.transpose(0, 2, 1)).astype(np.float32)

    pekdT = np.ascontiguousarray((pe_k[0:33] - pe_k[0]).T).astype(np.float32)
    pek0 = np.ascontiguousarray(pe_k[0][:, None]).astype(np.float32)
    dv = np.ascontiguousarray(pe_v[1:33] - pe_v[0]).astype(np.float32)
    pev0 = np.broadcast_to(pe_v[0], (128, D)).astype(np.float32).copy()

    dpad = np.zeros((DPAD_ROWS, DPAD_W), np.float32)
    dpad[1:, MAXP:159] = -1e9
    dpad16 = dpad.reshape(-1).astype(ml_dtypes.bfloat16)
    zscr = np.zeros((VSCR_N,), ml_dtypes.bfloat16)

    in_maps = []
    for c in range(NCORES):
        sl = slice(c * BHPC, (c + 1) * BHPC)
        in_maps.append(
            {
                "qt": qT[sl],
                "kt": kT[sl],
                "v": v[sl],
                "pekdT": pekdT,
                "pek0": pek0,
                "dv": dv,
                "pev0": pev0,
                "dpad_init": dpad16,
                "zscr": zscr,
            }
        )
    return in_maps


def assemble(results):
    output = np.empty((BH, S, D), np.float32)
    p_attn = np.zeros((BH, S, S), np.float32)
    for c in range(NCORES):
        r = results[c]
        out_c = np.asarray(r["out"], np.float32)
        pp = np.asarray(r["p_pack"], np.float32).reshape(BHPC, PPACK_PER_BH)
        for i in range(BHPC):
            bh = c * BHPC + i
            output[bh] = out_c[i]
            for qt in range(NQ):
                Wq = W_OF[qt]
                blk = pp[i, BASE_OF[qt] : BASE_OF[qt] + 128 * Wq].reshape(128, Wq)
                p_attn[bh, qt * 128 : (qt + 1) * 128, 0:Wq] = blk
    return output.reshape(B, H, S, D), p_attn.reshape(B, H, S, S)


def kernel(query, key, value, pe_k, pe_v):
    from concourse.bass_utils import run_bass_kernel_spmd

    if "nc" not in _CACHE:
        _CACHE["nc"] = _build_nc()
    nc = _CACHE["nc"]
    in_maps = host_prep(
        np.asarray(query), np.asarray(key), np.asarray(value),
        np.asarray(pe_k), np.asarray(pe_v),
    )
    res = run_bass_kernel_spmd(nc, in_maps, list(range(NCORES)))
    return assemble(res.results)


# revision 7
# speedup vs baseline: 1.3217x; 1.3217x over previous
"""Relative-position causal attention (B=4,H=16,S=1024,D=64) on 8 TRN2 NeuronCores.

Sharding: batch*heads (64) split 8 per core; pe tables replicated; no collectives.

Per (bh, q-block qt):
  scores[128, W] = (Q/8) @ (K + pe_k[0])^T in fp32 PSUM  (W=(qt+1)*128)
  qrel[128,33]   = (Q/8) @ (pe_k[r]-pe_k[0])^T
  band + causal mask injected via a DRAM stride trick: write qrel buckets
  1..32 as rows of stride 512 (bf16), read [128, 128/256] at row stride 511
  => per-row diagonal shift; the scratch's static padding supplies -1e9
  (upper triangle) and 0 (distant region, where pe_k[0] is already absorbed
  into K).
  exp on ACT (no max subtraction; fp32-safe) -> E bf16 + fp32 rowsum;
  p = E * (1/rowsum) in bf16; DMA the row to the packed lower-tri output.
  PV: for kt<=qt read P^T tiles via XBAR-transposed DMA of the packed output;
  matmul-accumulate P^T.T @ (V + pe_v[0]). Value band: skew-write the p row's
  diagonal vicinity to a stride-160 scratch, XBAR-transposed read, slice
  [0:32] -> one extra matmul against (pe_v[1..32]-pe_v[0]).

Host assembles full p_attn (upper tri = exact 0) and output; returns
(output, p_attn) like the reference.
"""

import numpy as np

B, H, S, D = 4, 16, 1024, 64
MAXP = 32
NCORES = 8
BH = B * H
BHPC = BH // NCORES
NQ = S // 128
W_OF = [(qt + 1) * 128 for qt in range(NQ)]
BASE_OF = [128 * 128 * (qt * (qt + 1) // 2) for qt in range(NQ)]
PPACK_PER_BH = 128 * 128 * (NQ * (NQ + 1) // 2)  # 589824

DPAD_ROWS, DPAD_W = 129, 512
DPAD_N = DPAD_ROWS * DPAD_W  # one qt region
DPAD_ALL = NQ * DPAD_N  # per double-buffer: 8 regions, one per qt
VSCR_ST = 160
VSCR_ROWS = 130  # padded so [128,128] transposed reads stay in bounds
VSCR_N = VSCR_ROWS * VSCR_ST

_CACHE = {}


def _split_sync_waits(nc, max_waits=1):
    """This walrus build rejects >1 sync wait per instruction; move excess
    waits onto preceding NoOp carriers on the same engine."""
    from concourse import mybir

    for bb in nc.main_func.blocks:
        il = bb.instructions
        i = 0
        while i < len(il):
            ins = il[i]
            si = ins.sync_info
            if si is not None and si.on_wait is not None and len(si.on_wait) > max_waits:
                waits = list(si.on_wait)
                keep = waits[-max_waits:]
                excess = waits[:-max_waits]
                pos = i
                for j in range(0, len(excess), max_waits):
                    chunk = excess[j : j + max_waits]
                    nop = mybir.InstNoOp(name=f"{ins.name}_wsplit{j}", ins=[], outs=[])
                    nop.engine = ins.engine
                    nop.sync_info = mybir.SyncInfo(on_wait=chunk, on_update=[])
                    il.insert(pos, nop)
                    pos += 1
                    i += 1
                ins.sync_info = mybir.SyncInfo(
                    on_wait=keep, on_update=list(si.on_update or [])
                )
            i += 1


def _build_nc(debug=False):
    import contextlib

    import concourse.bass as bass
    import concourse.tile as tile
    from concourse import mybir

    dt = mybir.dt
    f32, bf16 = dt.float32, dt.bfloat16
    AP = bass.AP

    if debug:
        nc = bass.Bass(target_bir_lowering=False, debug=True)
    else:
        nc = bass.Bass()

    qt_in = nc.declare_dram_parameter("qt", [BHPC, D, S], f32, isOutput=False)
    kt_in = nc.declare_dram_parameter("kt", [BHPC, D, S], f32, isOutput=False)
    v_in = nc.declare_dram_parameter("v", [BHPC, S, D], f32, isOutput=False)
    pekd_in = nc.declare_dram_parameter("pekdT", [D, 33], f32, isOutput=False)
    pek0_in = nc.declare_dram_parameter("pek0", [D, 1], f32, isOutput=False)
    dv_in = nc.declare_dram_parameter("dv", [MAXP, D], f32, isOutput=False)
    pev0_in = nc.declare_dram_parameter("pev0", [128, D], f32, isOutput=False)
    dpad_in = nc.declare_dram_parameter("dpad_init", [DPAD_N], bf16, isOutput=False)
    zscr_in = nc.declare_dram_parameter("zscr", [VSCR_N], bf16, isOutput=False)

    ppack = nc.declare_dram_parameter(
        "p_pack", [BHPC * PPACK_PER_BH], bf16, isOutput=True
    )
    out_o = nc.declare_dram_parameter("out", [BHPC, S, D], f32, isOutput=True)

    dpad = nc.dram_tensor("dpad", [2 * DPAD_ALL], bf16)
    vscr0 = nc.dram_tensor("vscr0", [2 * VSCR_N], bf16)
    vscrp = nc.dram_tensor("vscrp", [NQ * VSCR_N], bf16)

    with tile.TileContext(nc) as tc:
        ctx = contextlib.ExitStack()
        consts = ctx.enter_context(tc.tile_pool(name="consts", bufs=1))
        qk = ctx.enter_context(tc.tile_pool(name="qk", bufs=2))
        v0p = ctx.enter_context(tc.tile_pool(name="v0p", bufs=2))
        rows = ctx.enter_context(tc.tile_pool(name="rows", bufs=3))
        small = ctx.enter_context(tc.tile_pool(name="small", bufs=4))
        ptp = ctx.enter_context(tc.tile_pool(name="ptp", bufs=8))
        psum = ctx.enter_context(tc.tile_pool(name="psum", bufs=3, space="PSUM"))
        psum2 = ctx.enter_context(tc.tile_pool(name="psum2", bufs=2, space="PSUM"))

        # one-time constants (gpsimd = bulk/setup DMA engine)
        pekd32 = consts.tile([D, 33], f32)
        nc.gpsimd.dma_start(out=pekd32[:], in_=pekd_in[:, :])
        pekd16 = consts.tile([D, 33], bf16)
        nc.vector.tensor_copy(out=pekd16[:], in_=pekd32[:])

        pek0 = consts.tile([D, 1], f32)
        nc.gpsimd.dma_start(out=pek0[:], in_=pek0_in[:, :])

        dv32 = consts.tile([MAXP, D], f32)
        nc.gpsimd.dma_start(out=dv32[:], in_=dv_in[:, :])
        dv16 = consts.tile([MAXP, D], bf16)
        nc.vector.tensor_copy(out=dv16[:], in_=dv32[:])

        pev0 = consts.tile([128, D], f32)
        nc.gpsimd.dma_start(out=pev0[:], in_=pev0_in[:, :])

        for buf in range(2):
            for r in range(NQ):
                nc.gpsimd.dma_start(
                    out=AP(tensor=dpad, offset=buf * DPAD_ALL + r * DPAD_N, ap=[[DPAD_W, DPAD_ROWS], [1, DPAD_W]]),
                    in_=AP(tensor=dpad_in, offset=0, ap=[[DPAD_W, DPAD_ROWS], [1, DPAD_W]]),
                )
            nc.gpsimd.dma_start(
                out=AP(tensor=vscr0, offset=buf * VSCR_N, ap=[[VSCR_ST, VSCR_ROWS], [1, VSCR_ST]]),
                in_=AP(tensor=zscr_in, offset=0, ap=[[VSCR_ST, VSCR_ROWS], [1, VSCR_ST]]),
            )
        for r in range(NQ):
            nc.gpsimd.dma_start(
                out=AP(tensor=vscrp, offset=r * VSCR_N, ap=[[VSCR_ST, VSCR_ROWS], [1, VSCR_ST]]),
                in_=AP(tensor=zscr_in, offset=0, ap=[[VSCR_ST, VSCR_ROWS], [1, VSCR_ST]]),
            )

        step = 0
        for ibh in range(BHPC):
            dbuf = ibh % 2
            q32 = qk.tile([D, S], f32, tag="q32")
            nc.gpsimd.dma_start(out=q32[:], in_=qt_in[ibh, :, :])
            q16 = qk.tile([D, S], bf16, tag="q16")
            nc.vector.tensor_scalar_mul(q16[:], q32[:], 1.0 / np.sqrt(D))

            k32 = qk.tile([D, S], f32, tag="k32")
            nc.gpsimd.dma_start(out=k32[:], in_=kt_in[ibh, :, :])
            k16 = qk.tile([D, S], bf16, tag="k16")
            nc.vector.tensor_scalar_add(k16[:], k32[:], pek0[:])

            # all 8 V tiles in one DMA: [1024,64] -> [128, 8, 64]
            v32a = v0p.tile([128, NQ, D], f32, tag="v32a")
            nc.gpsimd.dma_start(
                out=v32a[:],
                in_=AP(tensor=v_in, offset=ibh * S * D, ap=[[D, 128], [128 * D, NQ], [1, D]]),
            )
            v0 = []
            for kt in range(NQ):
                v0t = v0p.tile([128, D], bf16, tag=f"v0_{kt}")
                nc.vector.tensor_add(v0t[:], v32a[:, kt, :], pev0[:])
                v0.append(v0t)

            # all 8 qrel matmuls upfront; collect band rows into qd_all
            qd_all = small.tile([128, NQ, MAXP], bf16, tag="qd_all")
            for qt in range(NQ):
                qrel = psum2.tile([128, 128], f32, tag="smallp")
                nc.tensor.matmul(
                    qrel[:, 64:97], q16[:, qt * 128 : (qt + 1) * 128], pekd16[:],
                    start=True, stop=True,
                )
                nc.vector.tensor_copy(out=qd_all[:, qt, :], in_=qrel[:, 65:97])
            # one skew write for all 8 qt regions; one batched read back
            nc.gpsimd.dma_start(
                out=AP(tensor=dpad, offset=dbuf * DPAD_ALL + DPAD_W, ap=[[DPAD_W, 128], [DPAD_N, NQ], [1, MAXP]]),
                in_=qd_all[:],
            )
            bb0 = small.tile([128, 128], bf16, tag="bb0")
            nc.gpsimd.dma_start(
                out=bb0[:],
                in_=AP(tensor=dpad, offset=dbuf * DPAD_ALL + DPAD_W + 31, ap=[[DPAD_W - 1, 128], [1, 128]]),
            )
            bb_all = small.tile([128, NQ - 1, 256], bf16, tag="bb_all")
            nc.gpsimd.dma_start(
                out=bb_all[:],
                in_=AP(
                    tensor=dpad,
                    offset=dbuf * DPAD_ALL + DPAD_N + DPAD_W - 97,
                    ap=[[DPAD_W - 1, 128], [DPAD_N, NQ - 1], [1, 256]],
                ),
            )

            osb_all = small.tile([128, NQ, D], f32, tag="osb_all")
            for qt in range(NQ):
                W = W_OF[qt]
                step += 1
                qsl = slice(qt * 128, (qt + 1) * 128)

                # scores
                sc = psum.tile([128, 1024], f32, tag="scores")
                for c0 in range(0, W, 512):
                    c1 = min(c0 + 512, W)
                    nc.tensor.matmul(
                        sc[:, c0:c1], q16[:, qsl], k16[:, c0:c1], start=True, stop=True
                    )
                if qt == 0:
                    nc.vector.tensor_add(sc[:, 0:128], sc[:, 0:128], bb0[:])
                else:
                    pr = slice((qt - 1) * 128, (qt + 1) * 128)
                    nc.vector.tensor_add(sc[:, pr], sc[:, pr], bb_all[:, qt - 1, :])

                # softmax
                e16 = rows.tile([128, 1024], bf16, tag="e16")
                rsum = small.tile([128, 1], f32, tag="rsum")
                nc.scalar.activation(
                    out=e16[:, :W],
                    in_=sc[:, :W],
                    func=mybir.ActivationFunctionType.Exp,
                    accum_out=rsum[:],
                )
                rinv = small.tile([128, 1], f32, tag="rinv")
                nc.vector.reciprocal(rinv[:], rsum[:])
                p16 = rows.tile([128, 1024], bf16, tag="p16")
                nc.vector.tensor_scalar_mul(p16[:, :W], e16[:, :W], rinv[:])

                pbase = ibh * PPACK_PER_BH + BASE_OF[qt]
                nc.sync.dma_start(
                    out=AP(tensor=ppack, offset=pbase, ap=[[W, 128], [1, W]]),
                    in_=p16[:, :W],
                )

                # PV via transposed readback of packed p tiles
                opsum = psum2.tile([128, 128], f32, tag="smallp")
                for kt in range(qt + 1):
                    pt = ptp.tile([128, 128], bf16, tag="pt")
                    nc.sync.dma_start(
                        out=pt[:],
                        in_=AP(tensor=ppack, offset=pbase + kt * 128, ap=[[W, 128], [1, 128]]),
                        transpose=True,
                    )
                    nc.tensor.matmul(
                        opsum[:, 0:D], pt[:], v0[kt][:], start=(kt == 0), stop=False
                    )

                # value band: skew-write p row vicinity, transposed read
                bwT = small.tile([128, 128], bf16, tag="bwT")
                if qt == 0:
                    nc.gpsimd.dma_start(
                        out=AP(tensor=vscr0, offset=dbuf * VSCR_N + VSCR_ST, ap=[[VSCR_ST, 128], [1, 128]]),
                        in_=p16[:, 0:128],
                    )
                    nc.sync.dma_start(
                        out=bwT[:],
                        in_=AP(tensor=vscr0, offset=dbuf * VSCR_N + VSCR_ST - 31, ap=[[VSCR_ST + 1, 128], [1, 128]]),
                        transpose=True,
                    )
                else:
                    nc.gpsimd.dma_start(
                        out=AP(tensor=vscrp, offset=qt * VSCR_N, ap=[[VSCR_ST, 128], [1, VSCR_ST]]),
                        in_=p16[:, (qt - 1) * 128 + 96 : (qt + 1) * 128],
                    )
                    nc.sync.dma_start(
                        out=bwT[:],
                        in_=AP(tensor=vscrp, offset=qt * VSCR_N + 1, ap=[[VSCR_ST + 1, 128], [1, 128]]),
                        transpose=True,
                    )
                nc.tensor.matmul(opsum[:, 0:D], bwT[0:MAXP, :], dv16[:], start=False, stop=True)

                nc.any.tensor_copy(out=osb_all[:, qt, :], in_=opsum[:, 0:D])

            nc.gpsimd.dma_start(
                out=AP(tensor=out_o, offset=ibh * S * D, ap=[[D, 128], [128 * D, NQ], [1, D]]),
                in_=osb_all[:],
            )

        ctx.close()

    if not debug:
        # only needed for the walrus compile path; CoreSim chokes on
        # instructions inserted outside the build hooks
        _split_sync_waits(nc)
    return nc


def host_prep(query, key, value, pe_k, pe_v):
    import ml_dtypes

    q = query.reshape(BH, S, D)
    k = key.reshape(BH, S, D)
    v = np.ascontiguousarray(value.reshape(BH, S, D)).astype(np.float32)
    qT = np.ascontiguousarray(q.transpose(0, 2, 1)).astype(np.float32)
    kT = np.ascontiguousarray(k.transpose(0, 2, 1)).astype(np.float32)

    pekdT = np.ascontiguousarray((pe_k[0:33] - pe_k[0]).T).astype(np.float32)
    pek0 = np.ascontiguousarray(pe_k[0][:, None]).astype(np.float32)
    dv = np.ascontiguousarray(pe_v[1:33] - pe_v[0]).astype(np.float32)
    pev0 = np.broadcast_to(pe_v[0], (128, D)).astype(np.float32).copy()

    dpad = np.zeros((DPAD_ROWS, DPAD_W), np.float32)
    dpad[1:, MAXP:159] = -1e9
    dpad16 = dpad.reshape(-1).astype(ml_dtypes.bfloat16)
    zscr = np.zeros((VSCR_N,), ml_dtypes.bfloat16)

    in_maps = []
    for c in range(NCORES):
        sl = slice(c * BHPC, (c + 1) * BHPC)
        in_maps.append(
            {
                "qt": qT[sl],
                "kt": kT[sl],
                "v": v[sl],
                "pekdT": pekdT,
                "pek0": pek0,
                "dv": dv,
                "pev0": pev0,
                "dpad_init": dpad16,
                "zscr": zscr,
            }
        )
    return in_maps


def assemble(results):
    output = np.empty((BH, S, D), np.float32)
    p_attn = np.zeros((BH, S, S), np.float32)
    for c in range(NCORES):
        r = results[c]
        out_c = np.asarray(r["out"], np.float32)
        pp = np.asarray(r["p_pack"], np.float32).reshape(BHPC, PPACK_PER_BH)
        for i in range(BHPC):
            bh = c * BHPC + i
            output[bh] = out_c[i]
            for qt in range(NQ):
                Wq = W_OF[qt]
                blk = pp[i, BASE_OF[qt] : BASE_OF[qt] + 128 * Wq].reshape(128, Wq)
                p_attn[bh, qt * 128 : (qt + 1) * 128, 0:Wq] = blk
    return output.reshape(B, H, S, D), p_attn.reshape(B, H, S, S)


def kernel(query, key, value, pe_k, pe_v):
    from concourse.bass_utils import run_bass_kernel_spmd

    if "nc" not in _CACHE:
        _CACHE["nc"] = _build_nc()
    nc = _CACHE["nc"]
    in_maps = host_prep(
        np.asarray(query), np.asarray(key), np.asarray(value),
        np.asarray(pe_k), np.asarray(pe_v),
    )
    res = run_bass_kernel_spmd(nc, in_maps, list(range(NCORES)))
    return assemble(res.results)


# revision 10
# speedup vs baseline: 2.5897x; 1.9594x over previous
"""Relative-position causal attention (B=4,H=16,S=1024,D=64) on 8 TRN2 NeuronCores.

Sharding: batch*heads (64) split 8 per core; pe tables replicated; no collectives.

Per (bh, q-block qt):
  scores[128, W] = (Q/8) @ (K + pe_k[0])^T in fp32 PSUM  (W=(qt+1)*128)
  Band + causal mask via a DRAM stride trick: write the 32 per-row relative
  scores (Q/8 @ (pe_k[r]-pe_k[0])^T buckets 1..32) as rows of stride 512,
  read back at row stride 511 => per-row diagonal shift; static -1e9/0
  padding supplies the causal mask / distant region.
  exp on ACT (no max subtraction) -> E bf16 + fp32 rowsum; p = E/rowsum bf16;
  DMA p row into a padded per-(bh,qt) DRAM layout (only the W valid cols).

Per bh (after all 8 p rows are out):
  PV with OUT TRANSPOSED: out^T[64, 1024] accumulates matmul(lhsT=V0[kt],
  rhs=PTcol(kt)) where PTcol(kt) = ONE XBAR-transposed DMA of the kt-th
  column slab of the padded p layout (uniform stride thanks to padding).
  Value band: p-row diagonal vicinities skew-written to a unified stride-160
  scratch; ONE XBAR-transposed read yields bwT_all[128, 8*128]; a single
  matmul(lhsT=Dv, rhs=bwT_all[0:32]) accumulates the band term.
  out^T -> SBUF -> DRAM [64, 1024]; host transposes (free).

Host assembles full p_attn (upper tri = exact 0) and output; returns
(output, p_attn) like the reference.
"""

import numpy as np

B, H, S, D = 4, 16, 1024, 64
MAXP = 32
NCORES = 8
BH = B * H
BHPC = BH // NCORES
NQ = S // 128
W_OF = [(qt + 1) * 128 for qt in range(NQ)]
PROW_N = 128 * S  # one padded p row block [128, 1024]
PPAD_PER_BH = NQ * PROW_N

DPAD_ROWS, DPAD_W = 129, 512
DPAD_N = DPAD_ROWS * DPAD_W  # one qt region
DPAD_ALL = NQ * DPAD_N

VSCR_ST = 160
VSCR_REG = (VSCR_ST + 1) * 128  # 20608: region stride so batched read lines up
VSCR_ALL = NQ * VSCR_REG  # 164864

_CACHE = {}


def _split_sync_waits(nc, max_waits=1):
    """This walrus build rejects >1 sync wait per instruction; move excess
    waits onto preceding NoOp carriers on the same engine."""
    from concourse import mybir

    for bb in nc.main_func.blocks:
        il = bb.instructions
        i = 0
        while i < len(il):
            ins = il[i]
            si = ins.sync_info
            if si is not None and si.on_wait is not None and len(si.on_wait) > max_waits:
                waits = list(si.on_wait)
                keep = waits[-max_waits:]
                excess = waits[:-max_waits]
                pos = i
                for j in range(0, len(excess), max_waits):
                    chunk = excess[j : j + max_waits]
                    nop = mybir.InstNoOp(name=f"{ins.name}_wsplit{j}", ins=[], outs=[])
                    nop.engine = ins.engine
                    nop.sync_info = mybir.SyncInfo(on_wait=chunk, on_update=[])
                    il.insert(pos, nop)
                    pos += 1
                    i += 1
                ins.sync_info = mybir.SyncInfo(
                    on_wait=keep, on_update=list(si.on_update or [])
                )
            i += 1


def _build_nc(debug=False):
    import contextlib

    import concourse.bass as bass
    import concourse.tile as tile
    from concourse import mybir

    dt = mybir.dt
    f32, bf16 = dt.float32, dt.bfloat16
    AP = bass.AP

    if debug:
        nc = bass.Bass(target_bir_lowering=False, debug=True)
    else:
        nc = bass.Bass()

    qt_in = nc.declare_dram_parameter("qt", [BHPC, D, S], f32, isOutput=False)
    kt_in = nc.declare_dram_parameter("kt", [BHPC, D, S], f32, isOutput=False)
    v_in = nc.declare_dram_parameter("v", [BHPC, S, D], f32, isOutput=False)
    pekd_in = nc.declare_dram_parameter("pekdT", [D, 33], f32, isOutput=False)
    pek0_in = nc.declare_dram_parameter("pek0", [D, 1], f32, isOutput=False)
    dv_in = nc.declare_dram_parameter("dv", [MAXP, D], f32, isOutput=False)
    pev0_in = nc.declare_dram_parameter("pev0", [128, D], f32, isOutput=False)
    dpad_in = nc.declare_dram_parameter("dpad_init", [DPAD_N], bf16, isOutput=False)
    zscr_in = nc.declare_dram_parameter("zscr", [VSCR_REG], bf16, isOutput=False)

    ppad = nc.declare_dram_parameter(
        "p_pad", [BHPC * PPAD_PER_BH], bf16, isOutput=True
    )
    out_o = nc.declare_dram_parameter("out", [BHPC, D, S], f32, isOutput=True)

    dpad = nc.dram_tensor("dpad", [2 * DPAD_ALL], bf16)
    vscr = nc.dram_tensor("vscr", [2 * VSCR_ALL], bf16)

    with tile.TileContext(nc) as tc:
        ctx = contextlib.ExitStack()
        consts = ctx.enter_context(tc.tile_pool(name="consts", bufs=1))
        qk = ctx.enter_context(tc.tile_pool(name="qk", bufs=2))
        v0p = ctx.enter_context(tc.tile_pool(name="v0p", bufs=2))
        rows = ctx.enter_context(tc.tile_pool(name="rows", bufs=3))
        small = ctx.enter_context(tc.tile_pool(name="small", bufs=4))
        ptp = ctx.enter_context(tc.tile_pool(name="ptp", bufs=2))
        psum = ctx.enter_context(tc.tile_pool(name="psum", bufs=2, space="PSUM"))
        psumo = ctx.enter_context(tc.tile_pool(name="psumo", bufs=1, space="PSUM"))

        # one-time constants
        pekd32 = consts.tile([D, 33], f32)
        nc.gpsimd.dma_start(out=pekd32[:], in_=pekd_in[:, :])
        pekd16 = consts.tile([D, 33], bf16)
        nc.vector.tensor_copy(out=pekd16[:], in_=pekd32[:])

        pek0 = consts.tile([D, 1], f32)
        nc.gpsimd.dma_start(out=pek0[:], in_=pek0_in[:, :])

        dv32 = consts.tile([MAXP, D], f32)
        nc.gpsimd.dma_start(out=dv32[:], in_=dv_in[:, :])
        dv16 = consts.tile([MAXP, D], bf16)
        nc.vector.tensor_copy(out=dv16[:], in_=dv32[:])

        pev0 = consts.tile([128, D], f32)
        nc.gpsimd.dma_start(out=pev0[:], in_=pev0_in[:, :])

        for buf in range(2):
            for r in range(NQ):
                nc.gpsimd.dma_start(
                    out=AP(tensor=dpad, offset=buf * DPAD_ALL + r * DPAD_N, ap=[[DPAD_W, DPAD_ROWS], [1, DPAD_W]]),
                    in_=AP(tensor=dpad_in, offset=0, ap=[[DPAD_W, DPAD_ROWS], [1, DPAD_W]]),
                )
                nc.gpsimd.dma_start(
                    out=AP(tensor=vscr, offset=buf * VSCR_ALL + r * VSCR_REG, ap=[[VSCR_ST, 128], [1, VSCR_ST]]),
                    in_=AP(tensor=zscr_in, offset=0, ap=[[VSCR_ST, 128], [1, VSCR_ST]]),
                )
                nc.gpsimd.dma_start(
                    out=AP(tensor=vscr, offset=buf * VSCR_ALL + r * VSCR_REG + 128 * VSCR_ST, ap=[[128, 1], [1, 128]]),
                    in_=AP(tensor=zscr_in, offset=0, ap=[[128, 1], [1, 128]]),
                )

        for ibh in range(BHPC):
            dbuf = ibh % 2
            q32 = qk.tile([D, S], f32, tag="q32")
            nc.gpsimd.dma_start(out=q32[:], in_=qt_in[ibh, :, :])
            q16 = qk.tile([D, S], bf16, tag="q16")
            nc.vector.tensor_scalar_mul(q16[:], q32[:], 1.0 / np.sqrt(D))

            k32 = qk.tile([D, S], f32, tag="k32")
            nc.gpsimd.dma_start(out=k32[:], in_=kt_in[ibh, :, :])
            k16 = qk.tile([D, S], bf16, tag="k16")
            nc.vector.tensor_scalar_add(k16[:], k32[:], pek0[:])

            v32a = v0p.tile([128, NQ, D], f32, tag="v32a")
            nc.gpsimd.dma_start(
                out=v32a[:],
                in_=AP(tensor=v_in, offset=ibh * S * D, ap=[[D, 128], [128 * D, NQ], [1, D]]),
            )
            v0 = []
            for kt in range(NQ):
                v0t = v0p.tile([128, D], bf16, tag=f"v0_{kt}")
                nc.vector.tensor_add(v0t[:], v32a[:, kt, :], pev0[:])
                v0.append(v0t)

            # all 8 qrel matmuls upfront; batched skew write + reads
            qd_all = small.tile([128, NQ, MAXP], bf16, tag="qd_all")
            for qt in range(NQ):
                qrel = psum.tile([128, 512], f32, tag="scores_s")
                nc.tensor.matmul(
                    qrel[:, 0:33], q16[:, qt * 128 : (qt + 1) * 128], pekd16[:],
                    start=True, stop=True,
                )
                nc.vector.tensor_copy(out=qd_all[:, qt, :], in_=qrel[:, 1:33])
            nc.gpsimd.dma_start(
                out=AP(tensor=dpad, offset=dbuf * DPAD_ALL + DPAD_W, ap=[[DPAD_W, 128], [DPAD_N, NQ], [1, MAXP]]),
                in_=qd_all[:],
            )
            bb0 = small.tile([128, 128], bf16, tag="bb0")
            nc.gpsimd.dma_start(
                out=bb0[:],
                in_=AP(tensor=dpad, offset=dbuf * DPAD_ALL + DPAD_W + 31, ap=[[DPAD_W - 1, 128], [1, 128]]),
            )
            bb_all = small.tile([128, NQ - 1, 256], bf16, tag="bb_all")
            nc.gpsimd.dma_start(
                out=bb_all[:],
                in_=AP(
                    tensor=dpad,
                    offset=dbuf * DPAD_ALL + DPAD_N + DPAD_W - 97,
                    ap=[[DPAD_W - 1, 128], [DPAD_N, NQ - 1], [1, 256]],
                ),
            )

            pbh = ibh * PPAD_PER_BH
            for qt in range(NQ):
                W = W_OF[qt]
                qsl = slice(qt * 128, (qt + 1) * 128)

                if W <= 512:
                    sc = psum.tile([128, 512], f32, tag="scores_s")
                else:
                    sc = psum.tile([128, 1024], f32, tag="scores_b")
                for c0 in range(0, W, 512):
                    c1 = min(c0 + 512, W)
                    nc.tensor.matmul(
                        sc[:, c0:c1], q16[:, qsl], k16[:, c0:c1], start=True, stop=True
                    )
                if qt == 0:
                    nc.vector.tensor_add(sc[:, 0:128], sc[:, 0:128], bb0[:])
                else:
                    pr = slice((qt - 1) * 128, (qt + 1) * 128)
                    nc.vector.tensor_add(sc[:, pr], sc[:, pr], bb_all[:, qt - 1, :])

                e16 = rows.tile([128, 1024], bf16, tag="e16")
                rsum = small.tile([128, 1], f32, tag="rsum")
                nc.scalar.activation(
                    out=e16[:, :W],
                    in_=sc[:, :W],
                    func=mybir.ActivationFunctionType.Exp,
                    accum_out=rsum[:],
                )
                rinv = small.tile([128, 1], f32, tag="rinv")
                nc.vector.reciprocal(rinv[:], rsum[:])
                p16 = rows.tile([128, 1024], bf16, tag="p16")
                nc.vector.tensor_scalar_mul(p16[:, :W], e16[:, :W], rinv[:])

                nc.sync.dma_start(
                    out=AP(tensor=ppad, offset=pbh + qt * PROW_N, ap=[[S, 128], [1, W]]),
                    in_=p16[:, :W],
                )

                # value-band skew write (region qt); batched transposed read later
                if qt == 0:
                    nc.gpsimd.dma_start(
                        out=AP(tensor=vscr, offset=dbuf * VSCR_ALL + 32, ap=[[VSCR_ST, 128], [1, 128]]),
                        in_=p16[:, 0:128],
                    )
                else:
                    nc.gpsimd.dma_start(
                        out=AP(tensor=vscr, offset=dbuf * VSCR_ALL + qt * VSCR_REG, ap=[[VSCR_ST, 128], [1, VSCR_ST]]),
                        in_=p16[:, (qt - 1) * 128 + 96 : (qt + 1) * 128],
                    )

            # ---- PV with out^T; one transposed read per kt ----
            outT = psumo.tile([D, S], f32, tag="outT")
            for kt in range(NQ):
                ncols = (NQ - kt) * 128
                ptc = ptp.tile([128, ncols], bf16, tag=f"ptc_{kt}")
                nc.sync.dma_start(
                    out=ptc[:],
                    in_=AP(tensor=ppad, offset=pbh + (kt * 128) * S + kt * 128, ap=[[S, ncols], [1, 128]]),
                    transpose=True,
                )
                if kt * 128 < 512:
                    nc.tensor.matmul(
                        outT[:, kt * 128 : 512], v0[kt][:], ptc[:, 0 : 512 - kt * 128],
                        start=(kt == 0), stop=False,
                    )
                    nc.tensor.matmul(
                        outT[:, 512:1024], v0[kt][:], ptc[:, 512 - kt * 128 :],
                        start=(kt == 0), stop=False,
                    )
                else:
                    nc.tensor.matmul(
                        outT[:, kt * 128 : 1024], v0[kt][:], ptc[:],
                        start=False, stop=False,
                    )

            bwT_all = small.tile([128, S], bf16, tag="bwT_all")
            nc.sync.dma_start(
                out=bwT_all[:],
                in_=AP(tensor=vscr, offset=dbuf * VSCR_ALL + 1, ap=[[VSCR_ST + 1, 1024], [1, 128]]),
                transpose=True,
            )
            nc.tensor.matmul(outT[:, 0:512], dv16[:], bwT_all[0:MAXP, 0:512], start=False, stop=True)
            nc.tensor.matmul(outT[:, 512:1024], dv16[:], bwT_all[0:MAXP, 512:1024], start=False, stop=True)

            outsb = small.tile([D, S], f32, tag="outsb")
            nc.any.tensor_copy(out=outsb[:], in_=outT[:])
            nc.gpsimd.dma_start(out=out_o[ibh, :, :], in_=outsb[:])

        ctx.close()

    if not debug:
        _split_sync_waits(nc)
    return nc


def host_prep(query, key, value, pe_k, pe_v):
    import ml_dtypes

    q = query.reshape(BH, S, D)
    k = key.reshape(BH, S, D)
    v = np.ascontiguousarray(value.reshape(BH, S, D)).astype(np.float32)
    qT = np.ascontiguousarray(q.transpose(0, 2, 1)).astype(np.float32)
    kT = np.ascontiguousarray(k.transpose(0, 2, 1)).astype(np.float32)

    pekdT = np.ascontiguousarray((pe_k[0:33] - pe_k[0]).T).astype(np.float32)
    pek0 = np.ascontiguousarray(pe_k[0][:, None]).astype(np.float32)
    dv = np.ascontiguousarray(pe_v[1:33] - pe_v[0]).astype(np.float32)
    pev0 = np.broadcast_to(pe_v[0], (128, D)).astype(np.float32).copy()

    dpad = np.zeros((DPAD_ROWS, DPAD_W), np.float32)
    dpad[1:, MAXP:159] = -1e9
    dpad16 = dpad.reshape(-1).astype(ml_dtypes.bfloat16)
    zscr = np.zeros((VSCR_REG,), ml_dtypes.bfloat16)

    in_maps = []
    for c in range(NCORES):
        sl = slice(c * BHPC, (c + 1) * BHPC)
        in_maps.append(
            {
                "qt": qT[sl],
                "kt": kT[sl],
                "v": v[sl],
                "pekdT": pekdT,
                "pek0": pek0,
                "dv": dv,
                "pev0": pev0,
                "dpad_init": dpad16,
                "zscr": zscr,
            }
        )
    return in_maps


def assemble(results):
    output = np.empty((BH, S, D), np.float32)
    p_attn = np.zeros((BH, S, S), np.float32)
    for c in range(NCORES):
        r = results[c]
        out_c = np.asarray(r["out"], np.float32)  # [BHPC, D, S]
        pp = np.asarray(r["p_pad"], np.float32).reshape(BHPC, NQ, 128, S)
        for i in range(BHPC):
            bh = c * BHPC + i
            output[bh] = out_c[i].T
            for qt in range(NQ):
                Wq = W_OF[qt]
                p_attn[bh, qt * 128 : (qt + 1) * 128, 0:Wq] = pp[i, qt, :, 0:Wq]
    return output.reshape(B, H, S, D), p_attn.reshape(B, H, S, S)


def kernel(query, key, value, pe_k, pe_v):
    from concourse.bass_utils import run_bass_kernel_spmd

    if "nc" not in _CACHE:
        _CACHE["nc"] = _build_nc()
    nc = _CACHE["nc"]
    in_maps = host_prep(
        np.asarray(query), np.asarray(key), np.asarray(value),
        np.asarray(pe_k), np.asarray(pe_v),
    )
    res = run_bass_kernel_spmd(nc, in_maps, list(range(NCORES)))
    return assemble(res.results)


# revision 12
# speedup vs baseline: 3.2036x; 1.2371x over previous
"""Relative-position causal attention (B=4,H=16,S=1024,D=64) on 8 TRN2 NeuronCores.

Sharding: batch*heads (64) split 8 per core; pe tables replicated; no collectives.

Per (bh, q-block qt):
  scores[128, W] = (Q/8) @ (K + pe_k[0])^T in fp32 PSUM  (W=(qt+1)*128)
  Band + causal mask via a DRAM stride trick: write the 32 per-row relative
  scores (Q/8 @ (pe_k[r]-pe_k[0])^T buckets 1..32) as rows of stride 512,
  read back at row stride 511 => per-row diagonal shift; static -1e9/0
  padding supplies the causal mask / distant region.
  exp on ACT (no max subtraction) -> E bf16 + fp32 rowsum; p = E/rowsum bf16;
  DMA p row into a padded per-(bh,qt) DRAM layout (only the W valid cols).

Per bh (after all 8 p rows are out):
  PV with OUT TRANSPOSED: out^T[64, 1024] accumulates matmul(lhsT=V0[kt],
  rhs=PTcol(kt)) where PTcol(kt) = ONE XBAR-transposed DMA of the kt-th
  column slab of the padded p layout (uniform stride thanks to padding).
  Value band: p-row diagonal vicinities skew-written to a unified stride-160
  scratch; ONE XBAR-transposed read yields bwT_all[128, 8*128]; a single
  matmul(lhsT=Dv, rhs=bwT_all[0:32]) accumulates the band term.
  out^T -> SBUF -> DRAM [64, 1024]; host transposes (free).

Host assembles full p_attn (upper tri = exact 0) and output; returns
(output, p_attn) like the reference.
"""

import numpy as np

B, H, S, D = 4, 16, 1024, 64
MAXP = 32
NCORES = 8
BH = B * H
BHPC = BH // NCORES
NQ = S // 128
W_OF = [(qt + 1) * 128 for qt in range(NQ)]
PROW_N = 128 * S  # one padded p row block [128, 1024]
PPAD_PER_BH = NQ * PROW_N

DPAD_ROWS, DPAD_W = 129, 512
DPAD_N = DPAD_ROWS * DPAD_W  # one qt region
DPAD_ALL = NQ * DPAD_N

VSCR_ST = 160
VSCR_REG = (VSCR_ST + 1) * 128  # 20608: region stride so batched read lines up
VSCR_ALL = NQ * VSCR_REG  # 164864

_CACHE = {}


def _split_sync_waits(nc, max_waits=1):
    """This walrus build rejects >1 sync wait per instruction; move excess
    waits onto preceding NoOp carriers on the same engine."""
    from concourse import mybir

    for bb in nc.main_func.blocks:
        il = bb.instructions
        i = 0
        while i < len(il):
            ins = il[i]
            si = ins.sync_info
            if si is not None and si.on_wait is not None and len(si.on_wait) > max_waits:
                waits = list(si.on_wait)
                keep = waits[-max_waits:]
                excess = waits[:-max_waits]
                pos = i
                for j in range(0, len(excess), max_waits):
                    chunk = excess[j : j + max_waits]
                    nop = mybir.InstNoOp(name=f"{ins.name}_wsplit{j}", ins=[], outs=[])
                    nop.engine = ins.engine
                    nop.sync_info = mybir.SyncInfo(on_wait=chunk, on_update=[])
                    il.insert(pos, nop)
                    pos += 1
                    i += 1
                ins.sync_info = mybir.SyncInfo(
                    on_wait=keep, on_update=list(si.on_update or [])
                )
            i += 1


def _build_nc(debug=False):
    import contextlib

    import concourse.bass as bass
    import concourse.tile as tile
    from concourse import mybir

    dt = mybir.dt
    f32, bf16 = dt.float32, dt.bfloat16
    AP = bass.AP

    if debug:
        nc = bass.Bass(target_bir_lowering=False, debug=True)
    else:
        nc = bass.Bass()

    qt_in = nc.declare_dram_parameter("qt", [BHPC, D, S], f32, isOutput=False)
    kt_in = nc.declare_dram_parameter("kt", [BHPC, D, S], f32, isOutput=False)
    v_in = nc.declare_dram_parameter("v", [BHPC, S, D], f32, isOutput=False)
    pekd_in = nc.declare_dram_parameter("pekdT", [D, 33], f32, isOutput=False)
    pek0_in = nc.declare_dram_parameter("pek0", [D, 1], f32, isOutput=False)
    dv_in = nc.declare_dram_parameter("dv", [MAXP, D], f32, isOutput=False)
    pev0_in = nc.declare_dram_parameter("pev0", [128, NQ * D], f32, isOutput=False)
    dpad_in = nc.declare_dram_parameter("dpad_init", [DPAD_ALL], bf16, isOutput=False)
    zscr_in = nc.declare_dram_parameter("zscr", [VSCR_ALL], bf16, isOutput=False)

    ppad = nc.declare_dram_parameter(
        "p_pad", [BHPC * PPAD_PER_BH], bf16, isOutput=True
    )
    out_o = nc.declare_dram_parameter("out", [BHPC, D, S], f32, isOutput=True)

    dpad = nc.dram_tensor("dpad", [2 * DPAD_ALL], bf16)
    vscr = nc.dram_tensor("vscr", [2 * VSCR_ALL], bf16)

    with tile.TileContext(nc) as tc:
        ctx = contextlib.ExitStack()
        consts = ctx.enter_context(tc.tile_pool(name="consts", bufs=1))
        qk = ctx.enter_context(tc.tile_pool(name="qk", bufs=2))
        v0p = ctx.enter_context(tc.tile_pool(name="v0p", bufs=2))
        rows = ctx.enter_context(tc.tile_pool(name="rows", bufs=3))
        small = ctx.enter_context(tc.tile_pool(name="small", bufs=4))
        ptp = ctx.enter_context(tc.tile_pool(name="ptp", bufs=2))
        psum = ctx.enter_context(tc.tile_pool(name="psum", bufs=2, space="PSUM"))
        psumo = ctx.enter_context(tc.tile_pool(name="psumo", bufs=1, space="PSUM"))

        # one-time constants
        pekd32 = consts.tile([D, 33], f32)
        nc.gpsimd.dma_start(out=pekd32[:], in_=pekd_in[:, :])
        pekd16 = consts.tile([D, 33], bf16)
        nc.vector.tensor_copy(out=pekd16[:], in_=pekd32[:])

        pek0 = consts.tile([D, 1], f32)
        nc.gpsimd.dma_start(out=pek0[:], in_=pek0_in[:, :])

        dv32 = consts.tile([MAXP, D], f32)
        nc.gpsimd.dma_start(out=dv32[:], in_=dv_in[:, :])
        dv16 = consts.tile([MAXP, D], bf16)
        nc.vector.tensor_copy(out=dv16[:], in_=dv32[:])

        pev0 = consts.tile([128, NQ * D], f32)
        nc.gpsimd.dma_start(out=pev0[:], in_=pev0_in[:, :])

        for buf in range(2):
            nc.gpsimd.dma_start(
                out=AP(tensor=dpad, offset=buf * DPAD_ALL, ap=[[DPAD_W, DPAD_ALL // DPAD_W], [1, DPAD_W]]),
                in_=AP(tensor=dpad_in, offset=0, ap=[[DPAD_W, DPAD_ALL // DPAD_W], [1, DPAD_W]]),
            )
            nc.gpsimd.dma_start(
                out=AP(tensor=vscr, offset=buf * VSCR_ALL, ap=[[VSCR_ST, VSCR_ALL // VSCR_ST], [1, VSCR_ST]]),
                in_=AP(tensor=zscr_in, offset=0, ap=[[VSCR_ST, VSCR_ALL // VSCR_ST], [1, VSCR_ST]]),
            )
            nc.gpsimd.dma_start(
                out=AP(tensor=vscr, offset=buf * VSCR_ALL + (VSCR_ALL // VSCR_ST) * VSCR_ST, ap=[[128, 1], [1, VSCR_ALL % VSCR_ST]]),
                in_=AP(tensor=zscr_in, offset=0, ap=[[128, 1], [1, VSCR_ALL % VSCR_ST]]),
            )

        for ibh in range(BHPC):
            dbuf = ibh % 2
            q32 = qk.tile([D, S], f32, tag="q32")
            nc.gpsimd.dma_start(out=q32[:], in_=qt_in[ibh, :, :])
            q16 = qk.tile([D, S], bf16, tag="q16")
            nc.vector.tensor_scalar_mul(q16[:], q32[:], 1.0 / np.sqrt(D))

            k32 = qk.tile([D, S], f32, tag="k32")
            nc.gpsimd.dma_start(out=k32[:], in_=kt_in[ibh, :, :])
            k16 = qk.tile([D, S], bf16, tag="k16")
            nc.vector.tensor_scalar_add(k16[:], k32[:], pek0[:])

            v32a = v0p.tile([128, NQ, D], f32, tag="v32a")
            nc.gpsimd.dma_start(
                out=v32a[:],
                in_=AP(tensor=v_in, offset=ibh * S * D, ap=[[D, 128], [128 * D, NQ], [1, D]]),
            )
            v0a = v0p.tile([128, NQ * D], bf16, tag="v0a")
            nc.vector.tensor_add(
                v0a[:], v32a[:].rearrange("p a b -> p (a b)"), pev0[:]
            )
            v0 = [v0a[:, kt * D : (kt + 1) * D] for kt in range(NQ)]

            # all 8 qrel matmuls upfront; batched skew write + reads
            qd_all = small.tile([128, NQ, MAXP], bf16, tag="qd_all")
            for qt in range(NQ):
                qrel = psum.tile([128, 512], f32, tag="scores_s")
                nc.tensor.matmul(
                    qrel[:, 0:33], q16[:, qt * 128 : (qt + 1) * 128], pekd16[:],
                    start=True, stop=True,
                )
                nc.vector.tensor_copy(out=qd_all[:, qt, :], in_=qrel[:, 1:33])
            nc.gpsimd.dma_start(
                out=AP(tensor=dpad, offset=dbuf * DPAD_ALL + DPAD_W, ap=[[DPAD_W, 128], [DPAD_N, NQ], [1, MAXP]]),
                in_=qd_all[:],
            )
            bb0 = small.tile([128, 128], bf16, tag="bb0")
            nc.gpsimd.dma_start(
                out=bb0[:],
                in_=AP(tensor=dpad, offset=dbuf * DPAD_ALL + DPAD_W + 31, ap=[[DPAD_W - 1, 128], [1, 128]]),
            )
            bb_all = small.tile([128, NQ - 1, 256], bf16, tag="bb_all")
            nc.gpsimd.dma_start(
                out=bb_all[:],
                in_=AP(
                    tensor=dpad,
                    offset=dbuf * DPAD_ALL + DPAD_N + DPAD_W - 97,
                    ap=[[DPAD_W - 1, 128], [DPAD_N, NQ - 1], [1, 256]],
                ),
            )

            pbh = ibh * PPAD_PER_BH
            p16a = rows.tile([128, NQ, S], bf16, tag="p16a")
            for qt in range(NQ):
                W = W_OF[qt]
                qsl = slice(qt * 128, (qt + 1) * 128)

                if W <= 512:
                    sc = psum.tile([128, 512], f32, tag="scores_s")
                else:
                    sc = psum.tile([128, 1024], f32, tag="scores_b")
                for c0 in range(0, W, 512):
                    c1 = min(c0 + 512, W)
                    nc.tensor.matmul(
                        sc[:, c0:c1], q16[:, qsl], k16[:, c0:c1], start=True, stop=True
                    )
                if qt == 0:
                    nc.vector.tensor_add(sc[:, 0:128], sc[:, 0:128], bb0[:])
                else:
                    pr = slice((qt - 1) * 128, (qt + 1) * 128)
                    nc.vector.tensor_add(sc[:, pr], sc[:, pr], bb_all[:, qt - 1, :])

                e16 = rows.tile([128, 1024], bf16, tag="e16")
                rsum = small.tile([128, 1], f32, tag="rsum")
                nc.scalar.activation(
                    out=e16[:, :W],
                    in_=sc[:, :W],
                    func=mybir.ActivationFunctionType.Exp,
                    accum_out=rsum[:],
                )
                rinv = small.tile([128, 1], f32, tag="rinv")
                nc.vector.reciprocal(rinv[:], rsum[:])
                nc.vector.tensor_scalar_mul(
                    p16a[:, qt, :W], e16[:, :W], rinv[:]
                )

                nc.sync.dma_start(
                    out=AP(tensor=ppad, offset=pbh + qt * PROW_N, ap=[[S, 128], [1, W]]),
                    in_=p16a[:, qt, :W],
                )

                if qt == 0:
                    nc.gpsimd.dma_start(
                        out=AP(tensor=vscr, offset=dbuf * VSCR_ALL + 32, ap=[[VSCR_ST, 128], [1, 128]]),
                        in_=p16a[:, 0, 0:128],
                    )

            base0 = p16a[:, 0, 0:160]
            nc.gpsimd.dma_start(
                out=AP(tensor=vscr, offset=dbuf * VSCR_ALL + VSCR_REG, ap=[[VSCR_ST, 128], [VSCR_REG, NQ - 1], [1, VSCR_ST]]),
                in_=AP(tensor=base0.tensor, offset=base0.offset + 1120, ap=[[NQ * S, 128], [S + 128, NQ - 1], [1, VSCR_ST]]),
            )

            # ---- PV with out^T; one transposed read per kt ----
            outT = psumo.tile([D, S], f32, tag="outT")
            for kt in range(NQ):
                ncols = (NQ - kt) * 128
                ptc = ptp.tile([128, ncols], bf16, tag=f"ptc_{kt}")
                nc.sync.dma_start(
                    out=ptc[:],
                    in_=AP(tensor=ppad, offset=pbh + (kt * 128) * S + kt * 128, ap=[[S, ncols], [1, 128]]),
                    transpose=True,
                )
                if kt * 128 < 512:
                    nc.tensor.matmul(
                        outT[:, kt * 128 : 512], v0[kt], ptc[:, 0 : 512 - kt * 128],
                        start=(kt == 0), stop=False,
                    )
                    nc.tensor.matmul(
                        outT[:, 512:1024], v0[kt], ptc[:, 512 - kt * 128 :],
                        start=(kt == 0), stop=False,
                    )
                else:
                    nc.tensor.matmul(
                        outT[:, kt * 128 : 1024], v0[kt], ptc[:],
                        start=False, stop=False,
                    )

            bwT_all = small.tile([128, S], bf16, tag="bwT_all")
            nc.sync.dma_start(
                out=bwT_all[:],
                in_=AP(tensor=vscr, offset=dbuf * VSCR_ALL + 1, ap=[[VSCR_ST + 1, 1024], [1, 128]]),
                transpose=True,
            )
            nc.tensor.matmul(outT[:, 0:512], dv16[:], bwT_all[0:MAXP, 0:512], start=False, stop=True)
            nc.tensor.matmul(outT[:, 512:1024], dv16[:], bwT_all[0:MAXP, 512:1024], start=False, stop=True)

            outsb = small.tile([D, S], f32, tag="outsb")
            nc.any.tensor_copy(out=outsb[:], in_=outT[:])
            nc.gpsimd.dma_start(out=out_o[ibh, :, :], in_=outsb[:])

        ctx.close()

    if not debug:
        _split_sync_waits(nc)
    return nc


def host_prep(query, key, value, pe_k, pe_v):
    import ml_dtypes

    q = query.reshape(BH, S, D)
    k = key.reshape(BH, S, D)
    v = np.ascontiguousarray(value.reshape(BH, S, D)).astype(np.float32)
    qT = np.ascontiguousarray(q.transpose(0, 2, 1)).astype(np.float32)
    kT = np.ascontiguousarray(k.transpose(0, 2, 1)).astype(np.float32)

    pekdT = np.ascontiguousarray((pe_k[0:33] - pe_k[0]).T).astype(np.float32)
    pek0 = np.ascontiguousarray(pe_k[0][:, None]).astype(np.float32)
    dv = np.ascontiguousarray(pe_v[1:33] - pe_v[0]).astype(np.float32)
    pev0 = np.tile(np.broadcast_to(pe_v[0], (128, D)), (1, NQ)).astype(np.float32)

    dpad = np.zeros((DPAD_ROWS, DPAD_W), np.float32)
    dpad[1:, MAXP:159] = -1e9
    dpad16 = np.tile(dpad.reshape(-1), NQ).astype(ml_dtypes.bfloat16)
    zscr = np.zeros((VSCR_ALL,), ml_dtypes.bfloat16)

    in_maps = []
    for c in range(NCORES):
        sl = slice(c * BHPC, (c + 1) * BHPC)
        in_maps.append(
            {
                "qt": qT[sl],
                "kt": kT[sl],
                "v": v[sl],
                "pekdT": pekdT,
                "pek0": pek0,
                "dv": dv,
                "pev0": pev0,
                "dpad_init": dpad16,
                "zscr": zscr,
            }
        )
    return in_maps


def assemble(results):
    output = np.empty((BH, S, D), np.float32)
    p_attn = np.zeros((BH, S, S), np.float32)
    for c in range(NCORES):
        r = results[c]
        out_c = np.asarray(r["out"], np.float32)  # [BHPC, D, S]
        pp = np.asarray(r["p_pad"], np.float32).reshape(BHPC, NQ, 128, S)
        for i in range(BHPC):
            bh = c * BHPC + i
            output[bh] = out_c[i].T
            for qt in range(NQ):
                Wq = W_OF[qt]
                p_attn[bh, qt * 128 : (qt + 1) * 128, 0:Wq] = pp[i, qt, :, 0:Wq]
    return output.reshape(B, H, S, D), p_attn.reshape(B, H, S, S)


def kernel(query, key, value, pe_k, pe_v):
    from concourse.bass_utils import run_bass_kernel_spmd

    if "nc" not in _CACHE:
        _CACHE["nc"] = _build_nc()
    nc = _CACHE["nc"]
    in_maps = host_prep(
        np.asarray(query), np.asarray(key), np.asarray(value),
        np.asarray(pe_k), np.asarray(pe_v),
    )
    res = run_bass_kernel_spmd(nc, in_maps, list(range(NCORES)))
    return assemble(res.results)
